# revision 29
# baseline (speedup 1.0000x reference)
"""Trainium2 Bass kernel for causal self-attention (nn_CausalSelfAttention).

Sharding: tensor-parallel on heads + data-parallel on batch.
8 cores = 2 batches x 4 head-groups (4 heads of 64 dims each per core).

Single fused pipeline:
  - All inputs/outputs bf16, host pre-swizzled so every DMA is contiguous
    with multi-KB per-partition lines; K/Q weights are d-tile-major so the
    startup loads split into need-ordered pieces (wk-dt0 + first half of
    x chunk 0 gate the first projection) and the first real matmul starts
    ~4us earlier than a whole-tile load order allows.
  - Attention is chunked by query-blocks of 512 (qc=0..3). Attention for
    chunk qc needs only K/Q/V of t-blocks <= 4qc+3, so projection of
    chunk qc+1 is emitted as PE "filler" between attention stages; the
    ScalarE exp of each stage ((N+352)/1.2ns) is the attention inner-loop
    bottleneck and hides under PE matmul work. ScalarE's deficit vs PE
    grows with qc (qc3: ~8.5us vs ~6.3us per head), so ALL output
    projections are held back and consumed as qc3 filler + drain cover
    instead of being spent in qc1/qc2 where chunk projections already
    saturate PE.
  - Scores use zero-padded per-head K (full 128-partition contraction):
    64-contraction matmuls at base_partition 64 (PE row-group packing
    of the head pair) HANG this hardware path, and zero-padding costs
    the same N cycles anyway.
  - Each stage packs two key-blocks of one head into a [128,1024] 2-bank
    PSUM tile exp'd by one ACTIVATE (ragged diagonal blocks pack
    back-to-back so no exp column is wasted). V carries a ones column so
    attV also accumulates the softmax denominator (softmax is
    unstabilized: |scores| <= ~8 for these inputs).
  - Tail: for the last head of qc3 the normalize is split by query
    columns — cols 0:256 are final one stage early (key-blocks 14/15
    only touch cols 256:512), so outproj of t-blocks 12-13 plus reserved
    qc2 outproj groups run while the final normalize chain
    (copy/broadcast/reciprocal/mul) completes; PE never idles long
    enough for the HAM clock gate to drop to half speed during the
    drain.
  - PE prewarm: dummy matmuls during the DMA wait so the HAM clock gate
    is at 8/8 when real work arrives; a tiny ScalarE consumer also
    pre-loads the exp table off the critical path.
Host sums the 4 partials per batch (fp64) and adds the bias.
"""
import sys

if "/opt/trn_rl_repo" not in sys.path:
    sys.path.insert(0, "/opt/trn_rl_repo")

from collections import deque

import ml_dtypes
import numpy as np

import concourse.bacc as bacc
import concourse.mybir as mybir
from concourse.bass import _add_dep_helper
import concourse.tile as tile
from concourse.bass_utils import run_bass_kernel_spmd

B, T, C, H, D = 2, 2048, 1024, 16, 64
NCORES = 8
HPC = H // (NCORES // B)  # 4 heads per core
CS = HPC * D              # 256 channel-shard
P = 128
CT = C // P               # 8 contraction tiles
DT = CS // P              # 2 d-tiles (head pairs)
NTB = T // P              # 16 t-blocks of 128
NQC = 4                   # query chunks of 512
F32 = mybir.dt.float32
F32R = mybir.dt.float32r
BF16 = mybir.dt.bfloat16
EXP = mybir.ActivationFunctionType.Exp

LAST_RESULTS = None  # BassKernelResults of the most recent kernel() call


def _stage_packs(qc):
    """Stages for one head of query-chunk qc. Each stage is a list of
    (jb, W, off): key-block jb, query width W (last W queries of the
    chunk, per causality), PSUM column offset. Two key-blocks per stage
    pack the ACTIVATE range [0, actw) contiguously; every matmul dst
    stays inside one 512-col bank."""
    stages = []
    for jb in range(0, 4 * qc, 2):
        stages.append(([(jb, 512, 0), (jb + 1, 512, 512)], 1024))
    d = 4 * qc
    stages.append(([(d, 512, 0), (d + 1, 384, 512)], 896))
    stages.append(([(d + 2, 256, 0), (d + 3, 128, 256)], 384))
    return stages


def _emit(nc, tc):
    # all inputs pre-swizzled on host: every DMA contiguous, big lines
    # x: [P, qc, ct, 512]; K/Q weights d-tile-major: [P, dt, ct, 128]
    xT = nc.dram_tensor("xT", [P, NQC, CT, 512], BF16,
                        kind="ExternalInput").ap()
    wqT = nc.dram_tensor("wqT", [P, DT, CT, P], BF16,
                         kind="ExternalInput").ap()
    wkT = nc.dram_tensor("wkT", [P, DT, CT, P], BF16,
                         kind="ExternalInput").ap()
    wvT = nc.dram_tensor("wvT", [P, CT, CS], BF16, kind="ExternalInput").ap()
    wpT = nc.dram_tensor("wpT", [P, DT, C], BF16, kind="ExternalInput").ap()
    mask = nc.dram_tensor("mask", [P, P], BF16, kind="ExternalInput").ap()
    out = nc.dram_tensor("out", [T, C], BF16, kind="ExternalOutput").ap()

    with (
        tc.tile_pool(name="persist", bufs=1) as pp,
        tc.tile_pool(name="work", bufs=1) as pw,
        tc.tile_pool(name="psum", bufs=1, space="PSUM") as px,
    ):
        # head pair dt stacked on partitions: h even 0-63, h odd 64-127
        qT = pp.tile([P, DT, T], BF16, name="qT")
        # zero-padded per-head K^T: head h's 64 rows live at partition
        # offset 64*(h%2); the other 64 partitions are zero, so scores
        # contract over the full 128 partitions (base-64 row-tiled
        # 64-contraction matmuls hang this hardware path)
        kz = [pp.tile([P, T], BF16, name=f"kz{h}") for h in range(HPC)]
        vp = pp.tile([P, NTB, HPC, D + 1], BF16, name="vp")
        yT = pp.tile([P, DT, T], BF16, name="yT")
        wp_sb = pp.tile([P, DT, C], BF16, name="wp_sb")
        mask_sb = pp.tile([P, P], BF16, name="mask_sb")
        w_sbs = {nm: pp.tile([P, DT, CT, P], BF16, name=f"{nm}_sb")
                 for nm in ("wk", "wq")}
        wv_sb = pp.tile([P, CT, CS], BF16, name="wv_sb")
        xc = [pp.tile([P, CT, 512], BF16, name=f"xc{i}") for i in range(NQC)]

        zerof = pp.tile([P, 512], F32, name="zerof")
        nc.vector.memset(zerof, 0.0)
        zr = pp.tile([P, 512], F32R, name="zr")
        nc.vector.tensor_copy(zr, zerof)
        onesf = pp.tile([P, D], F32, name="onesf")
        nc.vector.memset(onesf, 1.0)
        nc.vector.tensor_copy(
            vp[:, :, :, D], onesf.rearrange("p (a b) -> p a b", a=NTB)
        )  # ones columns -> attV also accumulates the softmax denominator
        # zero the dead half of each kz tile (overlaps the DMA wait)
        for h in range(HPC):
            dead = 0 if (h % 2) else D
            for tb in range(T // 512):
                nc.vector.tensor_copy(
                    kz[h][dead:dead + D, tb * 512:(tb + 1) * 512],
                    zerof[dead:dead + D, :],
                )

        # ---- input DMAs: all ACTIVE transfers share the core's HBM
        # bandwidth, so priority-order them: wk alone on the sync ring
        # and xc0+wq concurrent on the gpsimd ring (the first
        # projections' inputs get the bandwidth); everything later is
        # held behind explicit transfer deps in need-time order so it
        # never steals bandwidth from a load the pipeline is about to
        # block on.
        _wk = nc.sync.dma_start(w_sbs["wk"], wkT)
        nc.gpsimd.dma_start(xc[0], xT[:, 0])
        _wq = nc.gpsimd.dma_start(w_sbs["wq"], wqT)
        nc.scalar.dma_start(mask_sb, mask)
        _wv = nc.sync.dma_start(wv_sb, wvT)
        _add_dep_helper(_wv.ins, _wq.ins, sync=True,
                        reason="hold wv until critical loads done")
        _wp = nc.scalar.dma_start(wp_sb, wpT)
        _add_dep_helper(_wp.ins, _wv.ins, sync=True,
                        reason="wp needed only at qc3")
        prev = _wq
        for i in range(1, NQC):
            di = nc.gpsimd.dma_start(xc[i], xT[:, i])
            _add_dep_helper(di.ins, prev.ins, sync=True,
                            reason="hold x chunk until critical loads done")
            prev = di

        # dummy broadcast: loads the GpSimd ISA library (~7us) now instead
        # of at the first normalize; held past the weight DMAs so the
        # library-code DMA doesn't steal HBM bandwidth from startup loads
        libwarm = pw.tile([2, D], F32, name="libwarm")
        _lw = nc.gpsimd.partition_broadcast(libwarm, onesf[0:1, :])
        _add_dep_helper(_lw.ins, _wq.ins, sync=True,
                        reason="delay gpsimd lib load past critical loads")

        # PE prewarm: dummy matmuls on zeros while DMAs land, so the HAM
        # clock gate is ramping toward 8/8 when the projections start
        wps = px.tile([P, 1024], F32, tag="sps", bufs=2, name="warmps")
        for _ in range(10):
            nc.tensor.matmul(wps[:, 0:512], lhsT=zr[:, 0:P], rhs=zr,
                             start=True, stop=True)
        warmsink = pw.tile([1, 1], BF16, name="warmsink")
        nc.scalar.activation(warmsink, wps[0:1, 0:1], EXP)

        # ---------------- projection / outproj groups ----------------
        def proj_kq(nm, tcix, dt_):
            ts_ = slice(tcix * 512, (tcix + 1) * 512)
            ps = px.tile([P, 512], F32, tag="pj", bufs=2, name="pjps")
            for ct in range(CT):
                nc.tensor.matmul(
                    ps,
                    lhsT=w_sbs[nm][:, dt_, ct, :],
                    rhs=xc[tcix][:, ct, :],
                    start=(ct == 0),
                    stop=(ct == CT - 1),
                )
            # early chunks: split the PSUM drain across Vector+Scalar so a
            # congested Vector queue doesn't gate the pj-PSUM rotation
            # (Scalar has slack while qc0/qc1 exp is small)
            if nm == "wq":
                if tcix <= 1:
                    nc.vector.tensor_copy(
                        qT[:, dt_, tcix * 512:tcix * 512 + 256],
                        ps[:, 0:256])
                    nc.scalar.copy(
                        qT[:, dt_, tcix * 512 + 256:(tcix + 1) * 512],
                        ps[:, 256:512])
                else:
                    nc.vector.tensor_copy(qT[:, dt_, ts_], ps)
            else:
                if tcix <= 1:
                    nc.vector.tensor_copy(kz[2 * dt_][0:D, ts_], ps[0:D, :])
                    nc.scalar.copy(kz[2 * dt_ + 1][D:P, ts_], ps[D:P, :])
                else:
                    nc.vector.tensor_copy(kz[2 * dt_][0:D, ts_], ps[0:D, :])
                    nc.vector.tensor_copy(kz[2 * dt_ + 1][D:P, ts_],
                                          ps[D:P, :])

        def proj_v(tcix, g):
            tb = 4 * tcix + g
            ps = px.tile([P, 512], F32, tag="pj", bufs=2, name="pjps")
            for ct in range(CT):
                nc.tensor.matmul(
                    ps[:, 0:CS],
                    lhsT=xc[tcix][:, ct, g * P:(g + 1) * P],
                    rhs=wv_sb[:, ct, :],
                    start=(ct == 0),
                    stop=(ct == CT - 1),
                )
            vsrc = ps[:, 0:CS].rearrange("p (h d) -> p h d", h=HPC)
            if tcix <= 1:
                nc.vector.tensor_copy(vp[:, tb, 0:2, 0:D], vsrc[:, 0:2])
                nc.scalar.copy(vp[:, tb, 2:4, 0:D], vsrc[:, 2:4])
            else:
                nc.vector.tensor_copy(vp[:, tb, :, 0:D], vsrc)

        def chunk_groups(tcix):
            gs = []
            for nm in ("wk", "wq"):
                for dt_ in range(DT):
                    gs.append(lambda n=nm, d=dt_, t=tcix: proj_kq(n, t, d))
            for g in range(4):
                gs.append(lambda g_=g, t=tcix: proj_v(t, g_))
            return gs

        osbs = {}

        def outproj(tb, ob, split_cast=False, dep=None):
            # yT columns for a chunk are final once head 3's normalize
            # lands; project+store them while later attention runs.
            # bufs=6: the drain emits groups back-to-back, so the slot
            # rotation must not wait on out-DMA completion (~2us each)
            if ob == 0:
                osbs[tb] = pw.tile([P, C], BF16, tag="osb", bufs=8,
                                   name="osb")
            osb = osbs[tb]
            ps = px.tile([P, 512], F32, tag="pj", bufs=2, name="opps")
            for ct2 in range(DT):
                mm = nc.tensor.matmul(
                    ps,
                    lhsT=yT[:, ct2, tb * P:(tb + 1) * P],
                    rhs=wp_sb[:, ct2, ob * 512:(ob + 1) * 512],
                    start=(ct2 == 0),
                    stop=(ct2 == DT - 1),
                )
                if dep is not None and ct2 == 0:
                    # pin this group to the drain window: the greedy tile
                    # scheduler would otherwise hoist it into any earlier
                    # PE bubble, leaving nothing to cover the final
                    # normalize chain
                    _add_dep_helper(mm.ins, dep.ins, sync=True,
                                    reason="reserve outproj for the drain")
            dst = osb[:, ob * 512:(ob + 1) * 512]
            eng = nc.sync if tb % 2 == 0 else nc.gpsimd
            if split_cast:
                # drain phase: Scalar is idle; halve the PSUM-drain
                # latency and stream the output per half-row so the last
                # store pipeline is short
                nc.vector.tensor_copy(dst[:, 0:256], ps[:, 0:256])
                nc.scalar.copy(dst[:, 256:512], ps[:, 256:512])
                eng.dma_start(
                    out[tb * P:(tb + 1) * P, ob * 512:(ob + 1) * 512], dst)
                if ob == 1:
                    osbs.pop(tb)
            else:
                nc.vector.tensor_copy(dst, ps)
                if ob == 1:
                    eng.dma_start(out[tb * P:(tb + 1) * P, :], osbs.pop(tb))

        def outproj_big(tb, fine=False):
            # drain-phase outproj: both 512-col halves of a t-block in
            # one 4-bank PSUM tile (the attention "sps" slots are free by
            # then), casts split Vector/Scalar, per-half store. Twice the
            # work per pj-rotation grant keeps the drain MM-bound instead
            # of cast-latency-bound. fine=True (last t-block) streams
            # quarter-row pieces so the final store pipeline is short.
            osb = pw.tile([P, C], BF16, tag="osb", bufs=8, name="osb")
            ps = px.tile([P, 1024], F32, tag="sps", bufs=2, name="drps")
            for ob in range(2):
                for ct2 in range(DT):
                    nc.tensor.matmul(
                        ps[:, ob * 512:(ob + 1) * 512],
                        lhsT=yT[:, ct2, tb * P:(tb + 1) * P],
                        rhs=wp_sb[:, ct2, ob * 512:(ob + 1) * 512],
                        start=(ct2 == 0),
                        stop=(ct2 == DT - 1),
                    )
            eng = nc.sync if tb % 2 == 0 else nc.gpsimd
            row = out[tb * P:(tb + 1) * P, :]
            if fine:
                for pc in range(4):
                    cs_ = slice(pc * 256, (pc + 1) * 256)
                    e = nc.vector.tensor_copy if pc % 2 == 0 else \
                        nc.scalar.copy
                    e(osb[:, cs_], ps[:, cs_])
                    (nc.sync if pc % 2 == 0 else nc.gpsimd).dma_start(
                        row[:, cs_], osb[:, cs_])
            else:
                nc.vector.tensor_copy(osb[:, 0:512], ps[:, 0:512])
                eng.dma_start(row[:, 0:512], osb[:, 0:512])
                nc.scalar.copy(osb[:, 512:1024], ps[:, 512:1024])
                eng.dma_start(row[:, 512:1024], osb[:, 512:1024])

        # ---------------- attention ----------------
        psum_y = {}

        def emit_scores(st):
            h, qc, packs, actw = st["h"], st["qc"], st["packs"], st["actw"]
            dt_ = h // 2
            ps = px.tile([P, 1024], F32, tag="sps", bufs=2, name="sps")
            for jb, w, off in packs:
                qlo = qc * 512 + (512 - w)
                nc.tensor.matmul(
                    ps[:, off:off + w],
                    lhsT=kz[h][:, jb * P:(jb + 1) * P],
                    rhs=qT[:, dt_, qlo:qlo + w],
                    start=True,
                    stop=True,
                )
            strip = pw.tile([P, 1024], BF16, tag="att", bufs=6,
                            name=f"att_{h}_{qc}")
            nc.scalar.activation(strip[:, 0:actw], ps[:, 0:actw], EXP)
            # causal mask on each diagonal 128-block (first 128 cols of a
            # ragged segment)
            for jb, w, off in packs:
                if jb >= 4 * qc:
                    nc.vector.tensor_mul(
                        out=strip[:, off:off + P],
                        in0=strip[:, off:off + P],
                        in1=mask_sb,
                    )
            return strip

        def emit_attv(st, strip):
            h, qc, packs = st["h"], st["qc"], st["packs"]
            if st["first"]:
                psum_y[(h, qc)] = px.tile([D + 1, 512], F32, tag="ypsum",
                                          bufs=2, name=f"yps_{h}_{qc}")
            py_ = psum_y[(h, qc)]
            mm = None
            for jb, w, off in packs:
                mm = nc.tensor.matmul(
                    py_[:, 512 - w:512],
                    lhsT=vp[:, jb, h, :],
                    rhs=strip[:, off:off + w],
                    start=(st["first"] and off == 0),
                    stop=(st["last"] and jb == 4 * qc + 3),
                    skip_group_check=True,
                )
            if st["last"]:
                emit_norm(h, qc, 0, 512, pop=True)
            return mm

        def emit_norm(h, qc, c0, c1, pop):
            dt_ = h // 2
            ro = D * (h % 2)
            py_ = psum_y.pop((h, qc)) if pop else psum_y[(h, qc)]
            # denominator row -> broadcast across 64 partitions on the
            # (otherwise idle) GpSimd engine, fast reciprocal (~18 bits),
            # then scale y^T out of PSUM into bf16 yT
            srow = pw.tile([1, 512], F32, tag="srow", bufs=4, name="srow")
            nc.vector.tensor_copy(srow[:, c0:c1], py_[D:D + 1, c0:c1])
            sbc = pw.tile([D, 512], F32, tag="sbc", bufs=4, name="sbc")
            nc.gpsimd.partition_broadcast(sbc[:, c0:c1], srow[:, c0:c1])
            rsb = pw.tile([D, 512], F32, tag="rsb", bufs=4, name="rsb")
            nc.vector.reciprocal_approx_fast(out=rsb[:, c0:c1],
                                             in_=sbc[:, c0:c1])
            nc.vector.tensor_mul(
                out=yT[ro:ro + D, dt_, 512 * qc + c0:512 * qc + c1],
                in0=py_[0:D, c0:c1],
                in1=rsb[:, c0:c1],
            )

        # ---------------- fused pipeline ----------------
        # up front: chunk 0 K for both head pairs (needs only wk+xc0,
        # the first DMAs to land), two V t-blocks, then Q dt0 (the wq
        # DMA lands behind xc0 on its ring). attV runs 2 stages behind
        # scores; heads 2-3 start at stage 4.
        proj_kq("wk", 0, 0)
        proj_kq("wk", 0, 1)
        proj_kq("wq", 0, 0)
        proj_v(0, 0)
        proj_v(0, 1)

        stages = []
        for qc in range(NQC):
            for h in range(HPC):
                sl = _stage_packs(qc)
                for si, (packs, actw) in enumerate(sl):
                    stages.append(dict(qc=qc, h=h, packs=packs, actw=actw,
                                       first=(si == 0),
                                       last=(si == len(sl) - 1)))

        # fillers: chunk projections (hard deadline: before the stage
        # that reads their kz/qT columns hits the PE queue, else the
        # in-order PE queue deadlocks) and outproj groups. ScalarE's exp
        # deficit vs PE grows with qc, so outproj work is held for qc3 +
        # the drain.
        cfill = deque([lambda: proj_kq("wq", 0, 1)])
        cfill.extend([lambda g_=g: proj_v(0, g_) for g in range(2, 4)])
        cfill.extend(chunk_groups(1))
        cfill_at = {8: chunk_groups(2), 24: chunk_groups(3)}
        sfill = deque()

        final_attv = None
        pend = deque()  # software pipeline: attV runs 2 stages behind
        for i, st in enumerate(stages + [None, None]):
            if i in cfill_at:
                cfill.extend(cfill_at[i])
            if st is not None:
                strip = emit_scores(st)
                pend.append((st, strip))
            # filler cadence sets scheduler priority: chunk projections
            # own qc1/qc2 (outproj emitted there would contend for the
            # pj PSUM slots and the Vector queue right when the next
            # chunk's kz/qT writes are critical); outproj spends in qc3
            # where ScalarE exp outpaces PE attention work, keeping 6
            # groups for the drain.
            if st is not None and st["qc"] == 0:
                nfill = 2
            elif st is not None and st["qc"] in (1, 2):
                nfill = 1 if (cfill or len(sfill) > 12) else 0
            elif st is not None:  # qc3
                nfill = 1 if (cfill or len(sfill) > 6) else 0
            else:
                nfill = 0
            for _ in range(nfill):
                if cfill:
                    cfill.popleft()()
                elif sfill:
                    tb, ob = sfill.popleft()
                    outproj(tb, ob)
            if len(pend) > 2 or (st is None and pend):
                pst, pstrip = pend.popleft()
                mm = emit_attv(pst, pstrip)
                if pst["last"] and pst["h"] == HPC - 1:
                    if pst["qc"] < 3:
                        sfill.extend((tb, ob)
                                     for tb in range(4 * pst["qc"],
                                                     4 * pst["qc"] + 4)
                                     for ob in range(2))
                    else:
                        final_attv = mm

        # ---- drain: head 3's final attV lands just above; its normalize
        # chain (emitted by emit_attv) runs on GpSimd/Vector while the
        # leftover outproj groups — dep-pinned so the greedy scheduler
        # can't hoist them into earlier bubbles — keep PE busy, then
        # qc3's own outproj (which waits on that normalize) finishes.
        while sfill:
            tb, ob = sfill.popleft()
            outproj(tb, ob, split_cast=True, dep=final_attv)
        for tb in range(12, 16):
            outproj_big(tb, fine=True)


def build_program(num_devices=NCORES):
    nc = bacc.Bacc(
        "TRN2",
        target_bir_lowering=False,
        debug=False,
        num_devices=num_devices,
    )
    with tile.TileContext(nc) as tc:
        _emit(nc, tc)
    nc.compile()
    return nc


_PROGRAM = None


def _get_program():
    global _PROGRAM
    if _PROGRAM is None:
        _PROGRAM = build_program()
    return _PROGRAM


def _sw_w_dt(wT):
    # [C, CS] -> [P, DT, CT, P] contiguous (d-tile-major swizzle)
    return np.ascontiguousarray(
        wT.reshape(CT, P, DT, P).transpose(1, 2, 0, 3))


def _sw_w(wT):
    # [C, CS] -> [P, CT, CS] contiguous (partition-major swizzle)
    return np.ascontiguousarray(wT.reshape(CT, P, CS).transpose(1, 0, 2))


def make_in_maps(x, Wk, Wq, Wv, Wp):
    bf = ml_dtypes.bfloat16
    mask_np = np.triu(np.ones((P, P), np.float32)).astype(bf)
    in_maps = []
    for core in range(NCORES):
        b, g = divmod(core, HPC)
        rows = slice(CS * g, CS * (g + 1))
        xT = x[b].T.astype(bf)  # [C, T]
        xsw = np.ascontiguousarray(  # [C, T] -> [P, NQC, CT, 512]
            xT.reshape(CT, P, NQC, 512).transpose(1, 2, 0, 3))
        wpT = Wp[:, rows].T.astype(bf)  # [CS, C]
        wpsw = np.ascontiguousarray(
            wpT.reshape(DT, P, C).transpose(1, 0, 2))
        in_maps.append({
            "xT": xsw,
            "wqT": _sw_w_dt((Wq[rows].T * np.float32(0.125)).astype(bf)),
            "wkT": _sw_w_dt(Wk[rows].T.astype(bf)),
            "wvT": _sw_w(Wv[rows].T.astype(bf)),
            "wpT": wpsw,
            "mask": mask_np,
        })
    return in_maps


def kernel(x, Wk, Wq, Wv, Wp, bp):
    global LAST_RESULTS
    x = np.asarray(x, dtype=np.float32)
    Wk = np.asarray(Wk, dtype=np.float32)
    Wq = np.asarray(Wq, dtype=np.float32)
    Wv = np.asarray(Wv, dtype=np.float32)
    Wp = np.asarray(Wp, dtype=np.float32)
    bp = np.asarray(bp, dtype=np.float32)

    nc = _get_program()
    res = run_bass_kernel_spmd(
        nc, make_in_maps(x, Wk, Wq, Wv, Wp), core_ids=list(range(NCORES))
    )
    LAST_RESULTS = res

    out = np.zeros((B, T, C), np.float64)
    for core in range(NCORES):
        out[core // HPC] += np.asarray(res.results[core]["out"],
                                       dtype=np.float64)
    out += bp.astype(np.float64)[None, None, :]
    return out.astype(np.float32)


# revision 30
# speedup vs baseline: 1.0194x; 1.0194x over previous
"""Trainium2 Bass kernel for causal self-attention (nn_CausalSelfAttention).

Sharding: tensor-parallel on heads + data-parallel on batch.
8 cores = 2 batches x 4 head-groups (4 heads of 64 dims each per core).

Single fused pipeline:
  - All inputs/outputs bf16, host pre-swizzled so every DMA is contiguous
    with multi-KB per-partition lines; K/Q weights are d-tile-major so the
    startup loads split into need-ordered pieces (wk-dt0 + first half of
    x chunk 0 gate the first projection) and the first real matmul starts
    ~4us earlier than a whole-tile load order allows.
  - Attention is chunked by query-blocks of 512 (qc=0..3). Attention for
    chunk qc needs only K/Q/V of t-blocks <= 4qc+3, so projection of
    chunk qc+1 is emitted as PE "filler" between attention stages; the
    ScalarE exp of each stage ((N+352)/1.2ns) is the attention inner-loop
    bottleneck and hides under PE matmul work. ScalarE's deficit vs PE
    grows with qc (qc3: ~8.5us vs ~6.3us per head), so ALL output
    projections are held back and consumed as qc3 filler + drain cover
    instead of being spent in qc1/qc2 where chunk projections already
    saturate PE.
  - Scores use zero-padded per-head K (full 128-partition contraction):
    64-contraction matmuls at base_partition 64 (PE row-group packing
    of the head pair) HANG this hardware path, and zero-padding costs
    the same N cycles anyway.
  - Each stage packs two key-blocks of one head into a [128,1024] 2-bank
    PSUM tile exp'd by one ACTIVATE (ragged diagonal blocks pack
    back-to-back so no exp column is wasted). V carries a ones column so
    attV also accumulates the softmax denominator (softmax is
    unstabilized: |scores| <= ~8 for these inputs).
  - Tail: for the last head of qc3 the normalize is split by query
    columns — cols 0:256 are final one stage early (key-blocks 14/15
    only touch cols 256:512), so outproj of t-blocks 12-13 plus reserved
    qc2 outproj groups run while the final normalize chain
    (copy/broadcast/reciprocal/mul) completes; PE never idles long
    enough for the HAM clock gate to drop to half speed during the
    drain.
  - PE prewarm: dummy matmuls during the DMA wait so the HAM clock gate
    is at 8/8 when real work arrives; a tiny ScalarE consumer also
    pre-loads the exp table off the critical path.
Host sums the 4 partials per batch (fp64) and adds the bias.
"""
import sys

if "/opt/trn_rl_repo" not in sys.path:
    sys.path.insert(0, "/opt/trn_rl_repo")

from collections import deque

import ml_dtypes
import numpy as np

import concourse.bacc as bacc
import concourse.mybir as mybir
from concourse.bass import _add_dep_helper
import concourse.tile as tile
from concourse.bass_utils import run_bass_kernel_spmd

B, T, C, H, D = 2, 2048, 1024, 16, 64
NCORES = 8
HPC = H // (NCORES // B)  # 4 heads per core
CS = HPC * D              # 256 channel-shard
P = 128
CT = C // P               # 8 contraction tiles
DT = CS // P              # 2 d-tiles (head pairs)
NTB = T // P              # 16 t-blocks of 128
NQC = 4                   # query chunks of 512
F32 = mybir.dt.float32
F32R = mybir.dt.float32r
BF16 = mybir.dt.bfloat16
EXP = mybir.ActivationFunctionType.Exp

LAST_RESULTS = None  # BassKernelResults of the most recent kernel() call


def _stage_packs(qc):
    """Stages for one head of query-chunk qc. Each stage is a list of
    (jb, W, off): key-block jb, query width W (last W queries of the
    chunk, per causality), PSUM column offset. Two key-blocks per stage
    pack the ACTIVATE range [0, actw) contiguously; every matmul dst
    stays inside one 512-col bank."""
    stages = []
    for jb in range(0, 4 * qc, 2):
        stages.append(([(jb, 512, 0), (jb + 1, 512, 512)], 1024))
    d = 4 * qc
    stages.append(([(d, 512, 0), (d + 1, 384, 512)], 896))
    stages.append(([(d + 2, 256, 0), (d + 3, 128, 256)], 384))
    return stages


def _emit(nc, tc):
    # all inputs pre-swizzled on host: every DMA contiguous, big lines
    # x: [P, qc, ct, 512]; K/Q weights d-tile-major: [P, dt, ct, 128]
    xT = nc.dram_tensor("xT", [P, NQC, CT, 512], BF16,
                        kind="ExternalInput").ap()
    wqT = nc.dram_tensor("wqT", [P, DT, CT, P], BF16,
                         kind="ExternalInput").ap()
    wkT = nc.dram_tensor("wkT", [P, DT, CT, P], BF16,
                         kind="ExternalInput").ap()
    wvT = nc.dram_tensor("wvT", [P, CT, CS], BF16, kind="ExternalInput").ap()
    wpT = nc.dram_tensor("wpT", [P, DT, C], BF16, kind="ExternalInput").ap()
    mask = nc.dram_tensor("mask", [P, P], BF16, kind="ExternalInput").ap()
    out = nc.dram_tensor("out", [T, C], BF16, kind="ExternalOutput").ap()

    with (
        tc.tile_pool(name="persist", bufs=1) as pp,
        tc.tile_pool(name="work", bufs=1) as pw,
        tc.tile_pool(name="psum", bufs=1, space="PSUM") as px,
    ):
        # head pair dt stacked on partitions: h even 0-63, h odd 64-127
        qT = pp.tile([P, DT, T], BF16, name="qT")
        # zero-padded per-head K^T: head h's 64 rows live at partition
        # offset 64*(h%2); the other 64 partitions are zero, so scores
        # contract over the full 128 partitions (base-64 row-tiled
        # 64-contraction matmuls hang this hardware path)
        kz = [pp.tile([P, T], BF16, name=f"kz{h}") for h in range(HPC)]
        vp = pp.tile([P, NTB, HPC, D + 1], BF16, name="vp")
        yT = pp.tile([P, DT, T], BF16, name="yT")
        wp_sb = pp.tile([P, DT, C], BF16, name="wp_sb")
        mask_sb = pp.tile([P, P], BF16, name="mask_sb")
        w_sbs = {nm: pp.tile([P, DT, CT, P], BF16, name=f"{nm}_sb")
                 for nm in ("wk", "wq")}
        wv_sb = pp.tile([P, CT, CS], BF16, name="wv_sb")
        xc = [pp.tile([P, CT, 512], BF16, name=f"xc{i}") for i in range(NQC)]

        zerof = pp.tile([P, 512], F32, name="zerof")
        nc.vector.memset(zerof, 0.0)
        zr = pp.tile([P, 512], F32R, name="zr")
        nc.vector.tensor_copy(zr, zerof)
        onesf = pp.tile([P, D], F32, name="onesf")
        nc.vector.memset(onesf, 1.0)
        nc.vector.tensor_copy(
            vp[:, :, :, D], onesf.rearrange("p (a b) -> p a b", a=NTB)
        )  # ones columns -> attV also accumulates the softmax denominator
        # zero the dead half of each kz tile (overlaps the DMA wait)
        for h in range(HPC):
            dead = 0 if (h % 2) else D
            for tb in range(T // 512):
                nc.vector.tensor_copy(
                    kz[h][dead:dead + D, tb * 512:(tb + 1) * 512],
                    zerof[dead:dead + D, :],
                )

        # ---- input DMAs: all ACTIVE transfers share the core's HBM
        # bandwidth, so priority-order them: wk alone on the sync ring
        # and xc0+wq concurrent on the gpsimd ring (the first
        # projections' inputs get the bandwidth); everything later is
        # held behind explicit transfer deps in need-time order so it
        # never steals bandwidth from a load the pipeline is about to
        # block on.
        _wk = nc.sync.dma_start(w_sbs["wk"], wkT)
        nc.gpsimd.dma_start(xc[0], xT[:, 0])
        _wq = nc.gpsimd.dma_start(w_sbs["wq"], wqT)
        nc.scalar.dma_start(mask_sb, mask)
        _wv = nc.sync.dma_start(wv_sb, wvT)
        _add_dep_helper(_wv.ins, _wq.ins, sync=True,
                        reason="hold wv until critical loads done")
        _wp = nc.scalar.dma_start(wp_sb, wpT)
        _add_dep_helper(_wp.ins, _wv.ins, sync=True,
                        reason="wp needed only at qc3")
        prev = _wq
        for i in range(1, NQC):
            di = nc.gpsimd.dma_start(xc[i], xT[:, i])
            _add_dep_helper(di.ins, prev.ins, sync=True,
                            reason="hold x chunk until critical loads done")
            prev = di

        # dummy broadcast: loads the GpSimd ISA library (~7us) now instead
        # of at the first normalize; held past the weight DMAs so the
        # library-code DMA doesn't steal HBM bandwidth from startup loads
        libwarm = pw.tile([2, D], F32, name="libwarm")
        _lw = nc.gpsimd.partition_broadcast(libwarm, onesf[0:1, :])
        _add_dep_helper(_lw.ins, _wq.ins, sync=True,
                        reason="delay gpsimd lib load past critical loads")

        # PE prewarm: dummy matmuls on zeros while DMAs land, so the HAM
        # clock gate is ramping toward 8/8 when the projections start
        wps = px.tile([P, 1024], F32, tag="sps", bufs=2, name="warmps")
        for _ in range(10):
            nc.tensor.matmul(wps[:, 0:512], lhsT=zr[:, 0:P], rhs=zr,
                             start=True, stop=True)
        warmsink = pw.tile([1, 1], BF16, name="warmsink")
        nc.scalar.activation(warmsink, wps[0:1, 0:1], EXP)

        # ---------------- projection / outproj groups ----------------
        def proj_kq(nm, tcix, dt_):
            ts_ = slice(tcix * 512, (tcix + 1) * 512)
            ps = px.tile([P, 512], F32, tag="pj", bufs=2, name="pjps")
            for ct in range(CT):
                nc.tensor.matmul(
                    ps,
                    lhsT=w_sbs[nm][:, dt_, ct, :],
                    rhs=xc[tcix][:, ct, :],
                    start=(ct == 0),
                    stop=(ct == CT - 1),
                )
            # early chunks: split the PSUM drain across Vector+Scalar so a
            # congested Vector queue doesn't gate the pj-PSUM rotation
            # (Scalar has slack while qc0/qc1 exp is small)
            if nm == "wq":
                if tcix <= 1:
                    nc.vector.tensor_copy(
                        qT[:, dt_, tcix * 512:tcix * 512 + 256],
                        ps[:, 0:256])
                    nc.scalar.copy(
                        qT[:, dt_, tcix * 512 + 256:(tcix + 1) * 512],
                        ps[:, 256:512])
                else:
                    nc.vector.tensor_copy(qT[:, dt_, ts_], ps)
            else:
                if tcix <= 1:
                    nc.vector.tensor_copy(kz[2 * dt_][0:D, ts_], ps[0:D, :])
                    nc.scalar.copy(kz[2 * dt_ + 1][D:P, ts_], ps[D:P, :])
                else:
                    nc.vector.tensor_copy(kz[2 * dt_][0:D, ts_], ps[0:D, :])
                    nc.vector.tensor_copy(kz[2 * dt_ + 1][D:P, ts_],
                                          ps[D:P, :])

        def proj_v(tcix, g):
            tb = 4 * tcix + g
            ps = px.tile([P, 512], F32, tag="pj", bufs=2, name="pjps")
            for ct in range(CT):
                nc.tensor.matmul(
                    ps[:, 0:CS],
                    lhsT=xc[tcix][:, ct, g * P:(g + 1) * P],
                    rhs=wv_sb[:, ct, :],
                    start=(ct == 0),
                    stop=(ct == CT - 1),
                )
            vsrc = ps[:, 0:CS].rearrange("p (h d) -> p h d", h=HPC)
            if tcix <= 1:
                nc.vector.tensor_copy(vp[:, tb, 0:2, 0:D], vsrc[:, 0:2])
                nc.scalar.copy(vp[:, tb, 2:4, 0:D], vsrc[:, 2:4])
            else:
                nc.vector.tensor_copy(vp[:, tb, :, 0:D], vsrc)

        def chunk_groups(tcix):
            gs = []
            for nm in ("wk", "wq"):
                for dt_ in range(DT):
                    gs.append(lambda n=nm, d=dt_, t=tcix: proj_kq(n, t, d))
            for g in range(4):
                gs.append(lambda g_=g, t=tcix: proj_v(t, g_))
            return gs

        osbs = {}

        def outproj(tb, ob, split_cast=False, dep=None):
            # yT columns for a chunk are final once head 3's normalize
            # lands; project+store them while later attention runs.
            # bufs=6: the drain emits groups back-to-back, so the slot
            # rotation must not wait on out-DMA completion (~2us each)
            if ob == 0:
                osbs[tb] = pw.tile([P, C], BF16, tag="osb", bufs=8,
                                   name="osb")
            osb = osbs[tb]
            ps = px.tile([P, 512], F32, tag="pj", bufs=2, name="opps")
            for ct2 in range(DT):
                mm = nc.tensor.matmul(
                    ps,
                    lhsT=yT[:, ct2, tb * P:(tb + 1) * P],
                    rhs=wp_sb[:, ct2, ob * 512:(ob + 1) * 512],
                    start=(ct2 == 0),
                    stop=(ct2 == DT - 1),
                )
                if dep is not None and ct2 == 0:
                    # pin this group to the drain window: the greedy tile
                    # scheduler would otherwise hoist it into any earlier
                    # PE bubble, leaving nothing to cover the final
                    # normalize chain
                    _add_dep_helper(mm.ins, dep.ins, sync=True,
                                    reason="reserve outproj for the drain")
            dst = osb[:, ob * 512:(ob + 1) * 512]
            eng = nc.sync if tb % 2 == 0 else nc.gpsimd
            if split_cast:
                # drain phase: Scalar is idle; halve the PSUM-drain
                # latency and stream the output per half-row so the last
                # store pipeline is short
                nc.vector.tensor_copy(dst[:, 0:256], ps[:, 0:256])
                nc.scalar.copy(dst[:, 256:512], ps[:, 256:512])
                eng.dma_start(
                    out[tb * P:(tb + 1) * P, ob * 512:(ob + 1) * 512], dst)
                if ob == 1:
                    osbs.pop(tb)
            else:
                nc.vector.tensor_copy(dst, ps)
                if ob == 1:
                    eng.dma_start(out[tb * P:(tb + 1) * P, :], osbs.pop(tb))

        def outproj_big(tb, fine=False):
            # drain-phase outproj: both 512-col halves of a t-block in
            # one 4-bank PSUM tile (the attention "sps" slots are free by
            # then), casts split Vector/Scalar, per-half store. Twice the
            # work per pj-rotation grant keeps the drain MM-bound instead
            # of cast-latency-bound. fine=True (last t-block) streams
            # quarter-row pieces so the final store pipeline is short.
            osb = pw.tile([P, C], BF16, tag="osb", bufs=8, name="osb")
            ps = px.tile([P, 1024], F32, tag="sps", bufs=2, name="drps")
            for ob in range(2):
                for ct2 in range(DT):
                    nc.tensor.matmul(
                        ps[:, ob * 512:(ob + 1) * 512],
                        lhsT=yT[:, ct2, tb * P:(tb + 1) * P],
                        rhs=wp_sb[:, ct2, ob * 512:(ob + 1) * 512],
                        start=(ct2 == 0),
                        stop=(ct2 == DT - 1),
                    )
            eng = nc.sync if tb % 2 == 0 else nc.gpsimd
            row = out[tb * P:(tb + 1) * P, :]
            if fine:
                for pc in range(4):
                    cs_ = slice(pc * 256, (pc + 1) * 256)
                    e = nc.vector.tensor_copy if pc % 2 == 0 else \
                        nc.scalar.copy
                    e(osb[:, cs_], ps[:, cs_])
                    (nc.sync if pc % 2 == 0 else nc.gpsimd).dma_start(
                        row[:, cs_], osb[:, cs_])
            else:
                nc.vector.tensor_copy(osb[:, 0:512], ps[:, 0:512])
                eng.dma_start(row[:, 0:512], osb[:, 0:512])
                nc.scalar.copy(osb[:, 512:1024], ps[:, 512:1024])
                eng.dma_start(row[:, 512:1024], osb[:, 512:1024])

        # ---------------- attention ----------------
        psum_y = {}

        def emit_scores(st):
            h, qc, packs, actw = st["h"], st["qc"], st["packs"], st["actw"]
            dt_ = h // 2
            ps = px.tile([P, 1024], F32, tag="sps", bufs=2, name="sps")
            for jb, w, off in packs:
                qlo = qc * 512 + (512 - w)
                nc.tensor.matmul(
                    ps[:, off:off + w],
                    lhsT=kz[h][:, jb * P:(jb + 1) * P],
                    rhs=qT[:, dt_, qlo:qlo + w],
                    start=True,
                    stop=True,
                )
            strip = pw.tile([P, 1024], BF16, tag="att", bufs=6,
                            name=f"att_{h}_{qc}")
            nc.scalar.activation(strip[:, 0:actw], ps[:, 0:actw], EXP)
            # causal mask on each diagonal 128-block (first 128 cols of a
            # ragged segment)
            for jb, w, off in packs:
                if jb >= 4 * qc:
                    nc.vector.tensor_mul(
                        out=strip[:, off:off + P],
                        in0=strip[:, off:off + P],
                        in1=mask_sb,
                    )
            return strip

        def emit_attv(st, strip):
            h, qc, packs = st["h"], st["qc"], st["packs"]
            if st["first"]:
                psum_y[(h, qc)] = px.tile([D + 1, 512], F32, tag="ypsum",
                                          bufs=2, name=f"yps_{h}_{qc}")
            py_ = psum_y[(h, qc)]
            mm = None
            for jb, w, off in packs:
                mm = nc.tensor.matmul(
                    py_[:, 512 - w:512],
                    lhsT=vp[:, jb, h, :],
                    rhs=strip[:, off:off + w],
                    start=(st["first"] and off == 0),
                    stop=(st["last"] and jb == 4 * qc + 3),
                    skip_group_check=True,
                )
            if st["last"]:
                emit_norm(h, qc, 0, 512, pop=True)
            return mm

        def emit_norm(h, qc, c0, c1, pop):
            dt_ = h // 2
            ro = D * (h % 2)
            py_ = psum_y.pop((h, qc)) if pop else psum_y[(h, qc)]
            # denominator row -> broadcast across 64 partitions on the
            # (otherwise idle) GpSimd engine, fast reciprocal (~18 bits),
            # then scale y^T out of PSUM into bf16 yT
            srow = pw.tile([1, 512], F32, tag="srow", bufs=4, name="srow")
            nc.vector.tensor_copy(srow[:, c0:c1], py_[D:D + 1, c0:c1])
            sbc = pw.tile([D, 512], F32, tag="sbc", bufs=4, name="sbc")
            nc.gpsimd.partition_broadcast(sbc[:, c0:c1], srow[:, c0:c1])
            rsb = pw.tile([D, 512], F32, tag="rsb", bufs=4, name="rsb")
            nc.vector.reciprocal_approx_fast(out=rsb[:, c0:c1],
                                             in_=sbc[:, c0:c1])
            nc.vector.tensor_mul(
                out=yT[ro:ro + D, dt_, 512 * qc + c0:512 * qc + c1],
                in0=py_[0:D, c0:c1],
                in1=rsb[:, c0:c1],
            )

        # ---------------- fused pipeline ----------------
        # up front: chunk 0 K for both head pairs (needs only wk+xc0,
        # the first DMAs to land), two V t-blocks, then Q dt0 (the wq
        # DMA lands behind xc0 on its ring). attV runs 2 stages behind
        # scores; heads 2-3 start at stage 4.
        proj_kq("wk", 0, 0)
        proj_kq("wk", 0, 1)
        proj_kq("wq", 0, 0)
        proj_v(0, 0)
        proj_v(0, 1)

        stages = []
        for qc in range(NQC):
            for h in range(HPC):
                sl = _stage_packs(qc)
                for si, (packs, actw) in enumerate(sl):
                    stages.append(dict(qc=qc, h=h, packs=packs, actw=actw,
                                       first=(si == 0),
                                       last=(si == len(sl) - 1)))

        # fillers: chunk projections (hard deadline: before the stage
        # that reads their kz/qT columns hits the PE queue, else the
        # in-order PE queue deadlocks) and outproj groups. ScalarE's exp
        # deficit vs PE grows with qc, so outproj work is held for qc3 +
        # the drain.
        cfill = deque([lambda: proj_kq("wq", 0, 1)])
        cfill.extend([lambda g_=g: proj_v(0, g_) for g in range(2, 4)])
        cfill.extend(chunk_groups(1))
        cfill_at = {8: chunk_groups(2), 24: chunk_groups(3)}
        sfill = deque()

        final_attv = None
        pend = deque()  # software pipeline: attV runs 2 stages behind
        for i, st in enumerate(stages + [None, None]):
            if i in cfill_at:
                cfill.extend(cfill_at[i])
            if st is not None:
                strip = emit_scores(st)
                pend.append((st, strip))
            # filler cadence sets scheduler priority: chunk projections
            # own qc1/qc2 (outproj emitted there would contend for the
            # pj PSUM slots and the Vector queue right when the next
            # chunk's kz/qT writes are critical); outproj spends in qc3
            # where ScalarE exp outpaces PE attention work, keeping 6
            # groups for the drain.
            if st is not None and st["qc"] == 0:
                nfill = 2
            elif st is not None and st["qc"] in (1, 2):
                nfill = 1 if (cfill or len(sfill) > 12) else 0
            elif st is not None:  # qc3
                nfill = 1 if (cfill or len(sfill) > 6) else 0
            else:
                nfill = 0
            for _ in range(nfill):
                if cfill:
                    cfill.popleft()()
                elif sfill:
                    tb, ob = sfill.popleft()
                    outproj(tb, ob)
            if len(pend) > 2 or (st is None and pend):
                pst, pstrip = pend.popleft()
                mm = emit_attv(pst, pstrip)
                if pst["last"] and pst["h"] == HPC - 1:
                    if pst["qc"] < 3:
                        sfill.extend((tb, ob)
                                     for tb in range(4 * pst["qc"],
                                                     4 * pst["qc"] + 4)
                                     for ob in range(2))
                    else:
                        final_attv = mm

        # ---- drain: head 3's final attV lands just above; its normalize
        # chain (emitted by emit_attv) runs on GpSimd/Vector while the
        # leftover outproj groups — dep-pinned so the greedy scheduler
        # can't hoist them into earlier bubbles — keep PE busy, then
        # qc3's own outproj (which waits on that normalize) finishes.
        while sfill:
            tb, ob = sfill.popleft()
            outproj(tb, ob, split_cast=True, dep=final_attv)
        for tb in range(12, 16):
            outproj_big(tb, fine=(tb == 15))


def build_program(num_devices=NCORES):
    nc = bacc.Bacc(
        "TRN2",
        target_bir_lowering=False,
        debug=False,
        num_devices=num_devices,
    )
    with tile.TileContext(nc) as tc:
        _emit(nc, tc)
    nc.compile()
    return nc


_PROGRAM = None


def _get_program():
    global _PROGRAM
    if _PROGRAM is None:
        _PROGRAM = build_program()
    return _PROGRAM


def _sw_w_dt(wT):
    # [C, CS] -> [P, DT, CT, P] contiguous (d-tile-major swizzle)
    return np.ascontiguousarray(
        wT.reshape(CT, P, DT, P).transpose(1, 2, 0, 3))


def _sw_w(wT):
    # [C, CS] -> [P, CT, CS] contiguous (partition-major swizzle)
    return np.ascontiguousarray(wT.reshape(CT, P, CS).transpose(1, 0, 2))


def make_in_maps(x, Wk, Wq, Wv, Wp):
    bf = ml_dtypes.bfloat16
    mask_np = np.triu(np.ones((P, P), np.float32)).astype(bf)
    in_maps = []
    for core in range(NCORES):
        b, g = divmod(core, HPC)
        rows = slice(CS * g, CS * (g + 1))
        xT = x[b].T.astype(bf)  # [C, T]
        xsw = np.ascontiguousarray(  # [C, T] -> [P, NQC, CT, 512]
            xT.reshape(CT, P, NQC, 512).transpose(1, 2, 0, 3))
        wpT = Wp[:, rows].T.astype(bf)  # [CS, C]
        wpsw = np.ascontiguousarray(
            wpT.reshape(DT, P, C).transpose(1, 0, 2))
        in_maps.append({
            "xT": xsw,
            "wqT": _sw_w_dt((Wq[rows].T * np.float32(0.125)).astype(bf)),
            "wkT": _sw_w_dt(Wk[rows].T.astype(bf)),
            "wvT": _sw_w(Wv[rows].T.astype(bf)),
            "wpT": wpsw,
            "mask": mask_np,
        })
    return in_maps


def kernel(x, Wk, Wq, Wv, Wp, bp):
    global LAST_RESULTS
    x = np.asarray(x, dtype=np.float32)
    Wk = np.asarray(Wk, dtype=np.float32)
    Wq = np.asarray(Wq, dtype=np.float32)
    Wv = np.asarray(Wv, dtype=np.float32)
    Wp = np.asarray(Wp, dtype=np.float32)
    bp = np.asarray(bp, dtype=np.float32)

    nc = _get_program()
    res = run_bass_kernel_spmd(
        nc, make_in_maps(x, Wk, Wq, Wv, Wp), core_ids=list(range(NCORES))
    )
    LAST_RESULTS = res

    out = np.zeros((B, T, C), np.float64)
    for core in range(NCORES):
        out[core // HPC] += np.asarray(res.results[core]["out"],
                                       dtype=np.float64)
    out += bp.astype(np.float64)[None, None, :]
    return out.astype(np.float32)
